# revision 9
# baseline (speedup 1.0000x reference)
"""Trainium2 Bass kernel for nn_ComplexityAttention (GQA attention block).

Computation (B=1, S=2048, HID=2048, 16 Q heads / 4 KV heads, D=128):
  q/k/v = x @ W^T + mu @ Wm^T           (fused mu-guided projections)
  per-head RMSNorm on q, k; RoPE; causal GQA attention; out @ wo^T.

Sharding: tensor-parallel over heads across 8 NeuronCores. Core c owns
Q heads {2c, 2c+1} and KV head c//2 (KV work duplicated per core pair).
Each core produces a partial output (its heads' slice of wo applied),
host sums the 8 partials.

Device-side strategy (v2, fp8-DoubleRow projections):
  - Projections run on the PE in fp8e4m3 DoubleRow mode (256-deep
    contraction per matmul, 0.5 cycles/row): x is split hi+lo
    (x ~= xh + xl, both e4m3) and W is pre-scaled by 64 and split
    hi+lo; x@W ~= xh@Wh + xl@Wh + xh@Wl.  mu@Wm uses a single fp8
    term (the mu path is 10x smaller, quantization is negligible).
    The 64x weight scale cancels inside RMSNorm for q/k and is divided
    out during the V copy.
  - rstd = exp(-0.5*ln(var+eps)) so the Activation engine only ever
    uses {Square, Ln, Exp, Copy} - one table set, no table reloads.
  - Scores computed transposed S^T[kv, q] = K^T.T @ Q^T, softmax
    without max-subtraction, denominator via ones-vector matmul.
    Causal-diagonal blocks narrowed to the live q-range (no wasted
    columns); score matmuls emitted one kv-block ahead of the PV
    matmuls so the PE never waits on the Exp latency.
  - Output projection per 512-q chunk, staged to bf16 and written as
    one DMA per chunk; all inputs arrive as a handful of large DMAs
    (per-partition-contiguous layouts prepared on host).
"""

import sys

for _p in ("/opt/trn_rl_repo", "/root/.axon_site/_ro/trn_rl_repo"):
    if _p not in sys.path:
        sys.path.insert(0, _p)

import numpy as np
import ml_dtypes

import concourse.bass as bass
import concourse.bacc as bacc
import concourse.mybir as mybir
import concourse.tile as tile
from concourse.bass_utils import run_bass_kernel_spmd
from concourse.masks import make_identity

# Problem constants (hardcoded per contract)
B, S, HID = 1, 2048, 2048
NUM_HEADS, NUM_KV_HEADS, HEAD_DIM = 16, 4, 128
ROPE_THETA = 10000.0
EPS = 1e-6
N_CORES = 8

P = 128
KC = HID // P            # 16 contraction chunks of 128
KT = 8                   # contraction chunk-pairs (256 wide) for DoubleRow
SC = S // P              # 16 sequence chunks of 128
SCP = 2                  # s-chunks per projection pass
NPASS = SC // SCP        # 8 projection passes of 256 seq positions
QCH = 512                # attention q-chunk (one PSUM bank)
NQC = S // QCH           # 4
WSCALE = 64.0            # weight pre-scale for fp8 (cancels in norm / V copy)
QK_SCALE = 1.0 / float(np.sqrt(HEAD_DIM))

BF16 = mybir.dt.bfloat16
F32 = mybir.dt.float32
FP8 = mybir.dt.float8e4
NP_BF16 = ml_dtypes.bfloat16
NP_E4 = ml_dtypes.float8_e4m3

_PROGRAM = {}


def _build_program(repeats=1, dbg=False):
    """Build the per-core Bass/Tile program (identical on all 8 cores)."""
    AF = mybir.ActivationFunctionType
    OP = mybir.AluOpType
    DR = mybir.MatmulPerfMode.DoubleRow

    nc = bacc.Bacc(trn_type="TRN2", debug=False)

    # ---- DRAM I/O ----
    # x/mu in fp8, per-partition contiguous: [pass][p][kt][2][256]
    xh_d = nc.dram_tensor("xh", [NPASS, P, KT, 2, SCP * P], FP8, kind="ExternalInput")
    xl_d = nc.dram_tensor("xl", [NPASS, P, KT, 2, SCP * P], FP8, kind="ExternalInput")
    mu_d = nc.dram_tensor("mu8", [NPASS, P, KT, 2, SCP * P], FP8, kind="ExternalInput")
    # packed projection weights (64x pre-scaled): [p][kt][2][512] (q0|q1|k|v)
    wh_d = nc.dram_tensor("wh", [P, KT, 2, 512], FP8, kind="ExternalInput")
    wl_d = nc.dram_tensor("wl", [P, KT, 2, 512], FP8, kind="ExternalInput")
    wm_d = nc.dram_tensor("wm8", [P, KT, 2, 512], FP8, kind="ExternalInput")
    woT = nc.dram_tensor("woT", [2, P, HID], BF16, kind="ExternalInput")
    cosq = nc.dram_tensor("cosq", [SC, P, HEAD_DIM], F32, kind="ExternalInput")
    sinq = nc.dram_tensor("sinq", [SC, P, HEAD_DIM], F32, kind="ExternalInput")
    cosk = nc.dram_tensor("cosk", [SC, P, HEAD_DIM], F32, kind="ExternalInput")
    sink = nc.dram_tensor("sink", [SC, P, HEAD_DIM], F32, kind="ExternalInput")
    out_d = nc.dram_tensor("out", [KC, P, S], BF16, kind="ExternalOutput")
    if dbg:
        dbg_qt = [nc.dram_tensor(f"dbg_qt{h}", [P, S], BF16, kind="ExternalOutput")
                  for h in range(2)]
        dbg_kt = nc.dram_tensor("dbg_kt", [P, S], BF16, kind="ExternalOutput")
        dbg_v = nc.dram_tensor("dbg_v", [P, SC, HEAD_DIM], BF16, kind="ExternalOutput")
        dbg_at = [nc.dram_tensor(f"dbg_at{h}", [P, S], BF16, kind="ExternalOutput")
                  for h in range(2)]

    with tile.TileContext(nc) as tc:
        with (
            tc.tile_pool(name="persist", bufs=1) as persist,
            tc.tile_pool(name="stream", bufs=3) as stream,
            tc.tile_pool(name="tmp", bufs=6) as tmp,
            tc.tile_pool(name="small", bufs=8) as small,
            tc.tile_pool(name="expp", bufs=6) as expp,
            tc.tile_pool(name="rdbp", bufs=2) as rdbp,
            tc.tile_pool(name="ostage", bufs=2) as ostage,
            tc.tile_pool(name="ps_main", bufs=2, space="PSUM") as ps_main,
            tc.tile_pool(name="ps_scr", bufs=3, space="PSUM") as ps_scr,
            tc.tile_pool(name="ps_out", bufs=2, space="PSUM") as ps_out,
            tc.tile_pool(name="ps_den", bufs=1, space="PSUM") as ps_den,
        ):
            # ---- persistent SBUF tensors ----
            wh_sb = persist.tile([P, KT, 2, 512], FP8, name="wh_sb")
            wl_sb = persist.tile([P, KT, 2, 512], FP8, name="wl_sb")
            wm_sb = persist.tile([P, KT, 2, 512], FP8, name="wm_sb")
            wo_sb = persist.tile([P, 2, HID], BF16, name="wo_sb")
            cq_sb = persist.tile([P, SC, HEAD_DIM], F32, name="cq_sb")
            sq_sb = persist.tile([P, SC, HEAD_DIM], F32, name="sq_sb")
            ck_sb = persist.tile([P, SC, HEAD_DIM], F32, name="ck_sb")
            sk_sb = persist.tile([P, SC, HEAD_DIM], F32, name="sk_sb")
            qt_sb = [persist.tile([P, S], BF16, name=f"qt{h}_sb") for h in range(2)]
            kt_sb = persist.tile([P, S], BF16, name="kt_sb")
            v_sb = persist.tile([P, SC, HEAD_DIM], BF16, name="v_sb")
            attn_sb = [persist.tile([P, S], BF16, name=f"attn{c}_sb") for c in range(2)]
            ident = persist.tile([P, P], BF16, name="ident")
            ones_sb = persist.tile([P, 1], BF16, name="ones_sb")
            eps_sb = persist.tile([P, 1], F32, name="eps_sb")
            lnw_sb = persist.tile([P, 1], F32, name="lnw_sb")
            mask = persist.tile([P, P], BF16, name="mask")

            make_identity(nc, ident[:])
            nc.gpsimd.memset(ones_sb[:], 1.0)
            nc.gpsimd.memset(eps_sb[:], EPS)
            nc.gpsimd.memset(lnw_sb[:], -float(np.log(WSCALE)))
            # causal mask for 128x128 diagonal blocks (scores transposed:
            # rows=kv, cols=q; keep where q >= kv)
            nc.gpsimd.memset(mask[:], 1.0)
            nc.gpsimd.affine_select(
                out=mask[:],
                in_=mask[:],
                compare_op=mybir.AluOpType.is_ge,
                fill=0.0,
                base=0,
                pattern=[[1, P]],
                channel_multiplier=-1,
            )

            # ---- one-time input DMAs ----
            nc.scalar.dma_start(wh_sb[:], wh_d.ap())
            nc.scalar.dma_start(wl_sb[:], wl_d.ap())
            nc.scalar.dma_start(wm_sb[:], wm_d.ap())
            nc.scalar.dma_start(wo_sb[:], woT.ap().rearrange("c p o -> p c o"))
            nc.scalar.dma_start(cq_sb[:], cosq.ap().rearrange("s p d -> p s d"))
            nc.scalar.dma_start(sq_sb[:], sinq.ap().rearrange("s p d -> p s d"))
            nc.scalar.dma_start(ck_sb[:], cosk.ap().rearrange("s p d -> p s d"))
            nc.scalar.dma_start(sk_sb[:], sink.ap().rearrange("s p d -> p s d"))

            # (hidx within packed 512 cols, cos table, sin table, dest)
            norm_specs = [
                (2, ck_sb, sk_sb, kt_sb),
                (0, cq_sb, sq_sb, qt_sb[0]),
                (1, cq_sb, sq_sb, qt_sb[1]),
            ]

            def proj_pass(p):
                """Project s-columns [256p, 256p+256): fp8 DoubleRow matmuls,
                then RMSNorm+RoPE+transpose for q0/q1/k, V copy."""
                xh_t = stream.tile([P, KT, 2, SCP * P], FP8, tag="xh", name="xh_t")
                xl_t = stream.tile([P, KT, 2, SCP * P], FP8, tag="xl", name="xl_t")
                mu_t = stream.tile([P, KT, 2, SCP * P], FP8, tag="mu", name="mu_t")
                nc.sync.dma_start(xh_t[:], xh_d.ap()[p])
                nc.sync.dma_start(xl_t[:], xl_d.ap()[p])
                nc.sync.dma_start(mu_t[:], mu_d.ap()[p])
                pp = [
                    ps_main.tile([P, 512], F32, tag="main", name=f"pp{i}")
                    for i in range(SCP)
                ]
                for i2 in range(SCP):
                    sl = slice(i2 * P, (i2 + 1) * P)
                    for kt in range(KT):
                        nc.tensor.matmul(
                            pp[i2][:], xh_t[:, kt, :, sl], wh_sb[:, kt],
                            start=(kt == 0), stop=False, perf_mode=DR,
                        )
                        nc.tensor.matmul(
                            pp[i2][:], xl_t[:, kt, :, sl], wh_sb[:, kt],
                            start=False, stop=False, perf_mode=DR,
                        )
                        nc.tensor.matmul(
                            pp[i2][:], xh_t[:, kt, :, sl], wl_sb[:, kt],
                            start=False, stop=False, perf_mode=DR,
                        )
                        nc.tensor.matmul(
                            pp[i2][:], mu_t[:, kt, :, sl], wm_sb[:, kt],
                            start=False, stop=(kt == KT - 1), perf_mode=DR,
                        )
                # normalize + rope + transpose into [d, s] layout
                for i2 in range(SCP):
                    sc = p * SCP + i2
                    ps = pp[i2]
                    for hidx, c_sb, s_sb, dst in norm_specs:
                        off = hidx * P
                        sqv = tmp.tile([P, HEAD_DIM], F32, tag="sqv", name="sqv")
                        var = small.tile([P, 1], F32, tag="var", name="var")
                        nc.scalar.activation(
                            sqv[:], ps[:, off : off + P], AF.Square, accum_out=var[:]
                        )
                        # rstd = exp(-0.5*ln(mean(q^2)+eps)); the 64x weight
                        # scale cancels via 1/(HEAD_DIM*WSCALE^2)
                        lv = small.tile([P, 1], F32, tag="lv", name="lv")
                        nc.scalar.activation(
                            lv[:], var[:], AF.Ln,
                            scale=1.0 / (HEAD_DIM * WSCALE * WSCALE), bias=eps_sb[:],
                        )
                        # exp(-0.5*ln(var)+ln(1/WSCALE)) = (1/WSCALE)/rms: the
                        # extra bias undoes the weight pre-scale still in psum
                        rstd = small.tile([P, 1], F32, tag="rstd", name="rstd")
                        nc.scalar.activation(
                            rstd[:], lv[:], AF.Exp, scale=-0.5, bias=lnw_sb[:]
                        )
                        t1 = tmp.tile([P, HEAD_DIM], F32, tag="t1", name="t1")
                        nc.vector.scalar_tensor_tensor(
                            t1[:], ps[:, off : off + P], rstd[:], c_sb[:, sc, :],
                            op0=OP.mult, op1=OP.mult,
                        )
                        t2 = tmp.tile([P, HEAD_DIM], F32, tag="t2", name="t2")
                        nc.vector.scalar_tensor_tensor(
                            t2[:, 0:64], ps[:, off + 64 : off + P], rstd[:],
                            s_sb[:, sc, 0:64], op0=OP.mult, op1=OP.mult,
                        )
                        nc.vector.scalar_tensor_tensor(
                            t2[:, 64:P], ps[:, off : off + 64], rstd[:],
                            s_sb[:, sc, 64:P], op0=OP.mult, op1=OP.mult,
                        )
                        qsd = tmp.tile([P, HEAD_DIM], BF16, tag="qsd", name="qsd")
                        nc.vector.tensor_add(qsd[:], t1[:], t2[:])
                        tr = ps_scr.tile([P, P], BF16, tag="scr", name="tr")
                        nc.tensor.transpose(tr[:], qsd[:], ident[:])
                        nc.vector.tensor_copy(dst[:, sc * P : (sc + 1) * P], tr[:])
                    # V: copy (cast) with 1/WSCALE to undo the weight pre-scale
                    nc.scalar.activation(
                        v_sb[:, sc, :], ps[:, 384:512], AF.Copy, scale=1.0 / WSCALE
                    )

            def attention(qc):
                """Scores/exp/PV/den for q-chunk qc (columns [512qc,512qc+512)).
                Causal-narrowed; PV lags scores by one kv-block so the PE
                never waits on Exp."""
                jmax = 4 * qc + 3
                q0 = qc * QCH
                out_ps = [
                    ps_out.tile([P, QCH], F32, tag="out", name=f"out_ps{h}")
                    for h in range(2)
                ]
                den_t = ps_den.tile([64, QCH], F32, tag="den", name="den_t")
                pend = []

                def emit_pv(j, h, e, lo):
                    nc.tensor.matmul(
                        out_ps[h][:, lo:], v_sb[:, j, :], e[:, lo:],
                        start=(j == 0), stop=(j == jmax), skip_group_check=True,
                    )
                    nc.tensor.matmul(
                        den_t[32 * h : 32 * h + 1, lo:], ones_sb[:], e[:, lo:],
                        start=(j == 0), stop=(j == jmax), skip_group_check=True,
                    )

                for j in range(jmax + 1):
                    r = j - 4 * qc
                    lo = max(r, 0) * P
                    for h in range(2):
                        s_ps = ps_scr.tile([P, QCH], F32, tag="scr", name="s_ps")
                        nc.tensor.matmul(
                            s_ps[:, lo:],
                            kt_sb[:, j * P : (j + 1) * P],
                            qt_sb[h][:, q0 + lo : q0 + QCH],
                            start=True, stop=True,
                        )
                        e = expp.tile([P, QCH], BF16, tag="e", name="e")
                        nc.scalar.activation(
                            e[:, lo:], s_ps[:, lo:], AF.Exp, scale=QK_SCALE
                        )
                        if r >= 0:
                            nc.vector.tensor_mul(
                                e[:, lo : lo + P], e[:, lo : lo + P], mask[:]
                            )
                        pend.append((j, h, e, lo))
                    while len(pend) > 2:
                        emit_pv(*pend.pop(0))
                while pend:
                    emit_pv(*pend.pop(0))
                return out_ps, den_t

            def attn_div(qc, out_ps, den_t):
                q_sl = slice(qc * QCH, (qc + 1) * QCH)
                for h in range(2):
                    rd = small.tile([1, QCH], F32, tag="rd", name="rd")
                    nc.vector.reciprocal(rd[:], den_t[32 * h : 32 * h + 1, :])
                    rdb = rdbp.tile([P, QCH], F32, tag="rdb", name="rdb")
                    nc.gpsimd.partition_broadcast(rdb[:], rd[:])
                    nc.vector.tensor_mul(attn_sb[h][:, q_sl], out_ps[h][:], rdb[:])

            def do_wo(qc):
                """Output projection for q chunk qc; one output DMA per chunk."""
                q_sl = slice(qc * QCH, (qc + 1) * QCH)
                ob = ostage.tile([P, KC, QCH], BF16, tag="ob", name="ob")
                for oc in range(KC):
                    o_ps = ps_main.tile([P, QCH], F32, tag="main", name="o_ps")
                    nc.tensor.matmul(
                        o_ps[:], wo_sb[:, 0, oc * P : (oc + 1) * P],
                        attn_sb[0][:, q_sl], start=True, stop=False,
                    )
                    nc.tensor.matmul(
                        o_ps[:], wo_sb[:, 1, oc * P : (oc + 1) * P],
                        attn_sb[1][:, q_sl], start=False, stop=True,
                    )
                    nc.vector.tensor_copy(ob[:, oc, :], o_ps[:])
                nc.scalar.dma_start(
                    out_d.ap()[:, :, q_sl].rearrange("o p s -> p o s"), ob[:]
                )

            for rep in range(repeats):
                acc = {}
                for p in range(NPASS):
                    proj_pass(p)
                    if p % 2 == 1:
                        qc = (p - 1) // 2
                        acc[qc] = attention(qc)
                        attn_div(qc, *acc[qc])
                        if qc > 0:
                            do_wo(qc - 1)
                do_wo(NQC - 1)
                if dbg:
                    for h in range(2):
                        nc.sync.dma_start(dbg_qt[h].ap(), qt_sb[h][:])
                        nc.sync.dma_start(dbg_at[h].ap(), attn_sb[h][:])
                    nc.sync.dma_start(dbg_kt.ap(), kt_sb[:])
                    nc.sync.dma_start(dbg_v.ap(), v_sb[:])

    nc.compile()
    return nc


def _get_program(repeats=1):
    if repeats not in _PROGRAM:
        _PROGRAM[repeats] = _build_program(repeats)
    return _PROGRAM[repeats]


def _pack_x(a):
    """[S, HID] -> [NPASS, P, KT, 2, 256] with hid = kt*256 + i*128 + p."""
    return np.ascontiguousarray(
        a.reshape(NPASS, SCP * P, KT, 2, P).transpose(0, 4, 2, 3, 1)
    )


def _pack_w(a):
    """[HID, 512] -> [P, KT, 2, 512] with hid = kt*256 + i*128 + p."""
    return np.ascontiguousarray(a.reshape(KT, 2, P, 512).transpose(2, 0, 1, 3))


def _host_prepare(inputs):
    """Shard + lay out inputs for the 8 cores."""
    hs = np.asarray(inputs["hidden_states"], dtype=np.float32).reshape(S, HID)
    mu = np.asarray(inputs["mu_prev"], dtype=np.float32).reshape(S, HID)
    wq = np.asarray(inputs["wq"], dtype=np.float32)
    wk = np.asarray(inputs["wk"], dtype=np.float32)
    wv = np.asarray(inputs["wv"], dtype=np.float32)
    wo = np.asarray(inputs["wo"], dtype=np.float32)
    wmq = np.asarray(inputs["wmq"], dtype=np.float32)
    wmk = np.asarray(inputs["wmk"], dtype=np.float32)
    wmv = np.asarray(inputs["wmv"], dtype=np.float32)
    qw = np.asarray(inputs["q_norm_w"], dtype=np.float32)
    kw = np.asarray(inputs["k_norm_w"], dtype=np.float32)

    # hi/lo fp8 split of x; single fp8 for mu (its term is 10x smaller)
    xh8 = hs.astype(NP_E4)
    xl8 = (hs - xh8.astype(np.float32)).astype(NP_E4)
    mu8 = mu.astype(NP_E4)
    xh_p = _pack_x(xh8)
    xl_p = _pack_x(xl8)
    mu_p = _pack_x(mu8)

    # RoPE tables in [s, d] layout with rotate-half sign and norm weight baked in
    inv = 1.0 / (ROPE_THETA ** (np.arange(0, HEAD_DIM, 2, dtype=np.float32) / HEAD_DIM))
    ang = np.arange(S, dtype=np.float32)[:, None] * inv[None, :]  # [S, 64]
    emb = np.concatenate([ang, ang], axis=-1)  # [S, 128]
    cos_e = np.cos(emb)
    sin_e = np.sin(emb)
    sin_s = np.concatenate([-sin_e[:, :64], sin_e[:, 64:]], axis=-1)

    def tables(w):
        w_shift = np.concatenate([w[64:], w[:64]])
        cos_t = (cos_e * w[None, :]).astype(np.float32).reshape(SC, P, HEAD_DIM)
        sin_t = (sin_s * w_shift[None, :]).astype(np.float32).reshape(SC, P, HEAD_DIM)
        return np.ascontiguousarray(cos_t), np.ascontiguousarray(sin_t)

    cq, sq = tables(qw)
    ck, sk = tables(kw)

    in_maps = []
    for c in range(N_CORES):
        g = c // 2
        wq_s = wq[256 * c : 256 * (c + 1)]      # [256, HID]
        wmq_s = wmq[256 * c : 256 * (c + 1)]
        wk_s = wk[P * g : P * (g + 1)]          # [128, HID]
        wmk_s = wmk[P * g : P * (g + 1)]
        wv_s = wv[P * g : P * (g + 1)]
        wmv_s = wmv[P * g : P * (g + 1)]
        w_all = np.concatenate([wq_s.T, wk_s.T, wv_s.T], axis=1) * WSCALE  # [HID, 512]
        wm_all = np.concatenate([wmq_s.T, wmk_s.T, wmv_s.T], axis=1) * WSCALE
        wh8 = w_all.astype(NP_E4)
        wl8 = (w_all - wh8.astype(np.float32)).astype(NP_E4)
        wm8 = wm_all.astype(NP_E4)
        woT_c = wo[:, 256 * c : 256 * (c + 1)].T                     # [256, HID]
        in_maps.append(
            {
                "xh": xh_p,
                "xl": xl_p,
                "mu8": mu_p,
                "wh": _pack_w(wh8),
                "wl": _pack_w(wl8),
                "wm8": _pack_w(wm8),
                "woT": np.ascontiguousarray(woT_c).astype(NP_BF16).reshape(2, P, HID),
                "cosq": cq,
                "sinq": sq,
                "cosk": ck,
                "sink": sk,
            }
        )
    return in_maps


def run(inputs, trace=False):
    """Run the SPMD kernel; returns (full_output, exec_time_ns_or_None)."""
    nc = _get_program()
    in_maps = _host_prepare(inputs)
    res = run_bass_kernel_spmd(
        nc, in_maps, core_ids=list(range(N_CORES)), trace=trace
    )
    total = np.zeros((HID, S), dtype=np.float32)
    for c in range(N_CORES):
        total += res.results[c]["out"].astype(np.float32).reshape(HID, S)
    out = np.ascontiguousarray(total.T).reshape(B, S, HID).astype(np.float32)
    return out, res.exec_time_ns


def kernel(**inputs) -> np.ndarray:
    out, _ = run(inputs, trace=False)
    return out


# revision 13
# speedup vs baseline: 1.5145x; 1.5145x over previous
"""Trainium2 Bass kernel for nn_ComplexityAttention (GQA attention block).

Computation (B=1, S=2048, HID=2048, 16 Q heads / 4 KV heads, D=128):
  q/k/v = x @ W^T + mu @ Wm^T           (fused mu-guided projections)
  per-head RMSNorm on q, k; RoPE; causal GQA attention; out @ wo^T.

Sharding: tensor-parallel over heads across 8 NeuronCores. Core c owns
Q heads {2c, 2c+1} and KV head c//2 (KV work duplicated per core pair).
Each core produces a partial output (its heads' slice of wo applied),
host sums the 8 partials.

Device-side strategy (v2, fp8-DoubleRow projections):
  - Projections run on the PE in fp8e4m3 DoubleRow mode (256-deep
    contraction per matmul, 0.5 cycles/row): x is split hi+lo
    (x ~= xh + xl, both e4m3) and W is pre-scaled by 64 and split
    hi+lo; x@W ~= xh@Wh + xl@Wh + xh@Wl.  mu@Wm uses a single fp8
    term (the mu path is 10x smaller, quantization is negligible).
    The 64x weight scale cancels inside RMSNorm for q/k and is divided
    out during the V copy.
  - rstd = exp(-0.5*ln(var+eps)) so the Activation engine only ever
    uses {Square, Ln, Exp, Copy} - one table set, no table reloads.
  - Scores computed transposed S^T[kv, q] = K^T.T @ Q^T, softmax
    without max-subtraction, denominator via ones-vector matmul.
    Causal-diagonal blocks narrowed to the live q-range (no wasted
    columns); score matmuls emitted one kv-block ahead of the PV
    matmuls so the PE never waits on the Exp latency.
  - Output projection per 512-q chunk, staged to bf16 and written as
    one DMA per chunk; all inputs arrive as a handful of large DMAs
    (per-partition-contiguous layouts prepared on host).
"""

import sys

for _p in ("/opt/trn_rl_repo", "/root/.axon_site/_ro/trn_rl_repo"):
    if _p not in sys.path:
        sys.path.insert(0, _p)

import numpy as np
import ml_dtypes

import concourse.bass as bass
import concourse.bacc as bacc
import concourse.mybir as mybir
import concourse.tile as tile
from concourse.bass_utils import run_bass_kernel_spmd
from concourse.masks import make_identity

# Problem constants (hardcoded per contract)
B, S, HID = 1, 2048, 2048
NUM_HEADS, NUM_KV_HEADS, HEAD_DIM = 16, 4, 128
ROPE_THETA = 10000.0
EPS = 1e-6
N_CORES = 8

P = 128
KC = HID // P            # 16 contraction chunks of 128
KT = 8                   # contraction chunk-pairs (256 wide) for DoubleRow
SC = S // P              # 16 sequence chunks of 128
SCP = 2                  # s-chunks per projection pass
NPASS = SC // SCP        # 8 projection passes of 256 seq positions
QCH = 512                # attention q-chunk (one PSUM bank)
NQC = S // QCH           # 4
WSCALE = 64.0            # weight pre-scale for fp8 (cancels in norm / V copy)
QK_SCALE = 1.0 / float(np.sqrt(HEAD_DIM))

BF16 = mybir.dt.bfloat16
F32 = mybir.dt.float32
FP8 = mybir.dt.float8e4
NP_BF16 = ml_dtypes.bfloat16
NP_E4 = ml_dtypes.float8_e4m3

_PROGRAM = {}


def _build_program(repeats=1, dbg=False):
    """Build the per-core Bass/Tile program (identical on all 8 cores)."""
    AF = mybir.ActivationFunctionType
    OP = mybir.AluOpType
    DR = mybir.MatmulPerfMode.DoubleRow

    nc = bacc.Bacc(trn_type="TRN2", debug=False)

    # ---- DRAM I/O ----
    # x/mu in fp8, per-partition contiguous: [pass][p][kt][2][256]
    xh_d = nc.dram_tensor("xh", [NPASS, P, KT, 2, SCP * P], FP8, kind="ExternalInput")
    xl_d = nc.dram_tensor("xl", [NPASS, P, KT, 2, SCP * P], FP8, kind="ExternalInput")
    mu_d = nc.dram_tensor("mu8", [NPASS, P, KT, 2, SCP * P], FP8, kind="ExternalInput")
    # packed projection weights (64x pre-scaled): [p][kt][2][512] (q0|q1|k|v)
    wh_d = nc.dram_tensor("wh", [P, KT, 2, 512], FP8, kind="ExternalInput")
    wl_d = nc.dram_tensor("wl", [P, KT, 2, 512], FP8, kind="ExternalInput")
    wm_d = nc.dram_tensor("wm8", [P, KT, 2, 512], FP8, kind="ExternalInput")
    woT = nc.dram_tensor("woT", [2, P, HID], BF16, kind="ExternalInput")
    cosq = nc.dram_tensor("cosq", [SC, P, HEAD_DIM], F32, kind="ExternalInput")
    sinq = nc.dram_tensor("sinq", [SC, P, HEAD_DIM], F32, kind="ExternalInput")
    cosk = nc.dram_tensor("cosk", [SC, P, HEAD_DIM], F32, kind="ExternalInput")
    sink = nc.dram_tensor("sink", [SC, P, HEAD_DIM], F32, kind="ExternalInput")
    out_d = nc.dram_tensor("out", [KC, P, S], BF16, kind="ExternalOutput")
    if dbg:
        dbg_qt = [nc.dram_tensor(f"dbg_qt{h}", [P, S], BF16, kind="ExternalOutput")
                  for h in range(2)]
        dbg_kt = nc.dram_tensor("dbg_kt", [P, S], BF16, kind="ExternalOutput")
        dbg_v = nc.dram_tensor("dbg_v", [P, SC, HEAD_DIM], BF16, kind="ExternalOutput")
        dbg_at = [nc.dram_tensor(f"dbg_at{h}", [P, S], BF16, kind="ExternalOutput")
                  for h in range(2)]

    with tile.TileContext(nc) as tc:
        with (
            tc.tile_pool(name="persist", bufs=1) as persist,
            tc.tile_pool(name="stream", bufs=3) as stream,
            tc.tile_pool(name="tmp", bufs=6) as tmp,
            tc.tile_pool(name="small", bufs=8) as small,
            tc.tile_pool(name="expp", bufs=6) as expp,
            tc.tile_pool(name="rdbp", bufs=2) as rdbp,
            tc.tile_pool(name="ostage", bufs=2) as ostage,
            tc.tile_pool(name="ps_main", bufs=2, space="PSUM") as ps_main,
            tc.tile_pool(name="ps_scr", bufs=3, space="PSUM") as ps_scr,
            tc.tile_pool(name="ps_out", bufs=2, space="PSUM") as ps_out,
            tc.tile_pool(name="ps_den", bufs=1, space="PSUM") as ps_den,
        ):
            # ---- persistent SBUF tensors ----
            wh_sb = persist.tile([P, KT, 2, 512], FP8, name="wh_sb")
            wl_sb = persist.tile([P, KT, 2, 512], FP8, name="wl_sb")
            wm_sb = persist.tile([P, KT, 2, 512], FP8, name="wm_sb")
            wo_sb = persist.tile([P, 2, HID], BF16, name="wo_sb")
            cq_sb = persist.tile([P, SC, HEAD_DIM], F32, name="cq_sb")
            sq_sb = persist.tile([P, SC, HEAD_DIM], F32, name="sq_sb")
            ck_sb = persist.tile([P, SC, HEAD_DIM], F32, name="ck_sb")
            sk_sb = persist.tile([P, SC, HEAD_DIM], F32, name="sk_sb")
            qt_sb = [persist.tile([P, S], BF16, name=f"qt{h}_sb") for h in range(2)]
            kt_sb = persist.tile([P, S], BF16, name="kt_sb")
            v_sb = persist.tile([P, SC, HEAD_DIM], BF16, name="v_sb")
            attn_sb = [persist.tile([P, S], BF16, name=f"attn{c}_sb") for c in range(2)]
            ident = persist.tile([P, P], BF16, name="ident")
            ones_sb = persist.tile([P, 1], BF16, name="ones_sb")
            eps_sb = persist.tile([P, 1], F32, name="eps_sb")
            mask = persist.tile([P, P], BF16, name="mask")

            make_identity(nc, ident[:])
            nc.gpsimd.memset(ones_sb[:], 1.0)
            # bias for Sqrt: var/HEAD_DIM + WSCALE^2*eps = WSCALE^2*(mean+eps)
            nc.gpsimd.memset(eps_sb[:], EPS * WSCALE * WSCALE)
            # causal mask for 128x128 diagonal blocks (scores transposed:
            # rows=kv, cols=q; keep where q >= kv)
            nc.gpsimd.memset(mask[:], 1.0)
            nc.gpsimd.affine_select(
                out=mask[:],
                in_=mask[:],
                compare_op=mybir.AluOpType.is_ge,
                fill=0.0,
                base=0,
                pattern=[[1, P]],
                channel_multiplier=-1,
            )

            # ---- one-time input DMAs ----
            nc.scalar.dma_start(wh_sb[:], wh_d.ap())
            nc.scalar.dma_start(wl_sb[:], wl_d.ap())
            nc.scalar.dma_start(wm_sb[:], wm_d.ap())
            nc.scalar.dma_start(wo_sb[:], woT.ap().rearrange("c p o -> p c o"))
            nc.scalar.dma_start(cq_sb[:], cosq.ap().rearrange("s p d -> p s d"))
            nc.scalar.dma_start(sq_sb[:], sinq.ap().rearrange("s p d -> p s d"))
            nc.scalar.dma_start(ck_sb[:], cosk.ap().rearrange("s p d -> p s d"))
            nc.scalar.dma_start(sk_sb[:], sink.ap().rearrange("s p d -> p s d"))

            # (hidx within packed 512 cols, cos table, sin table, dest)
            norm_specs = [
                (2, ck_sb, sk_sb, kt_sb),
                (0, cq_sb, sq_sb, qt_sb[0]),
                (1, cq_sb, sq_sb, qt_sb[1]),
            ]

            def proj_pass(p):
                """Project s-columns [256p, 256p+256): fp8 DoubleRow matmuls,
                then RMSNorm+RoPE+transpose for q0/q1/k, V copy."""
                xh_t = stream.tile([P, KT, 2, SCP * P], FP8, tag="xh", name="xh_t")
                xl_t = stream.tile([P, KT, 2, SCP * P], FP8, tag="xl", name="xl_t")
                mu_t = stream.tile([P, KT, 2, SCP * P], FP8, tag="mu", name="mu_t")
                nc.sync.dma_start(xh_t[:], xh_d.ap()[p])
                nc.sync.dma_start(xl_t[:], xl_d.ap()[p])
                nc.sync.dma_start(mu_t[:], mu_d.ap()[p])
                pp = [
                    ps_main.tile([P, 512], F32, tag="main", name=f"pp{i}")
                    for i in range(SCP)
                ]
                for i2 in range(SCP):
                    sl = slice(i2 * P, (i2 + 1) * P)
                    for kt in range(KT):
                        nc.tensor.matmul(
                            pp[i2][:], xh_t[:, kt, :, sl], wh_sb[:, kt],
                            start=(kt == 0), stop=False, perf_mode=DR,
                        )
                        nc.tensor.matmul(
                            pp[i2][:], xl_t[:, kt, :, sl], wh_sb[:, kt],
                            start=False, stop=False, perf_mode=DR,
                        )
                        nc.tensor.matmul(
                            pp[i2][:], xh_t[:, kt, :, sl], wl_sb[:, kt],
                            start=False, stop=False, perf_mode=DR,
                        )
                        nc.tensor.matmul(
                            pp[i2][:], mu_t[:, kt, :, sl], wm_sb[:, kt],
                            start=False, stop=(kt == KT - 1), perf_mode=DR,
                        )
                # normalize + rope + transpose into [d, s] layout
                for i2 in range(SCP):
                    sc = p * SCP + i2
                    ps = pp[i2]
                    for hidx, c_sb, s_sb, dst in norm_specs:
                        off = hidx * P
                        sqv = tmp.tile([P, HEAD_DIM], F32, tag="sqv", name="sqv")
                        var = small.tile([P, 1], F32, tag="var", name="var")
                        nc.scalar.activation(
                            sqv[:], ps[:, off : off + P], AF.Square, accum_out=var[:]
                        )
                        # std64 = sqrt(var/HEAD_DIM + WSCALE^2*eps)
                        #       = WSCALE*sqrt(mean(q^2)+eps); its reciprocal
                        # also cancels the weight pre-scale still in psum
                        std = small.tile([P, 1], F32, tag="std", name="std")
                        nc.scalar.activation(
                            std[:], var[:], AF.Sqrt,
                            scale=1.0 / HEAD_DIM, bias=eps_sb[:],
                        )
                        rstd = small.tile([P, 1], F32, tag="rstd", name="rstd")
                        nc.vector.reciprocal(rstd[:], std[:])
                        t1 = tmp.tile([P, HEAD_DIM], F32, tag="t1", name="t1")
                        nc.vector.scalar_tensor_tensor(
                            t1[:], ps[:, off : off + P], rstd[:], c_sb[:, sc, :],
                            op0=OP.mult, op1=OP.mult,
                        )
                        t2 = tmp.tile([P, HEAD_DIM], F32, tag="t2", name="t2")
                        nc.vector.scalar_tensor_tensor(
                            t2[:, 0:64], ps[:, off + 64 : off + P], rstd[:],
                            s_sb[:, sc, 0:64], op0=OP.mult, op1=OP.mult,
                        )
                        nc.vector.scalar_tensor_tensor(
                            t2[:, 64:P], ps[:, off : off + 64], rstd[:],
                            s_sb[:, sc, 64:P], op0=OP.mult, op1=OP.mult,
                        )
                        qsd = tmp.tile([P, HEAD_DIM], BF16, tag="qsd", name="qsd")
                        nc.vector.tensor_add(qsd[:], t1[:], t2[:])
                        tr = ps_scr.tile([P, P], BF16, tag="scr", name="tr")
                        nc.tensor.transpose(tr[:], qsd[:], ident[:])
                        nc.vector.tensor_copy(dst[:, sc * P : (sc + 1) * P], tr[:])
                    # V: copy (cast) with 1/WSCALE to undo the weight pre-scale
                    nc.scalar.activation(
                        v_sb[:, sc, :], ps[:, 384:512], AF.Copy, scale=1.0 / WSCALE
                    )

            def attention(qc):
                """Scores/exp/PV/den for q-chunk qc (columns [512qc,512qc+512)).
                Causal-narrowed; PV lags scores by one kv-block so the PE
                never waits on Exp."""
                jmax = 4 * qc + 3
                q0 = qc * QCH
                out_ps = [
                    ps_out.tile([P, QCH], F32, tag="out", name=f"out_ps{h}")
                    for h in range(2)
                ]
                den_t = ps_den.tile([64, QCH], F32, tag="den", name="den_t")
                pend = []

                def emit_pv(j, h, e, lo):
                    nc.tensor.matmul(
                        out_ps[h][:, lo:], v_sb[:, j, :], e[:, lo:],
                        start=(j == 0), stop=(j == jmax), skip_group_check=True,
                    )
                    nc.tensor.matmul(
                        den_t[32 * h : 32 * h + 1, lo:], ones_sb[:], e[:, lo:],
                        start=(j == 0), stop=(j == jmax), skip_group_check=True,
                    )

                for j in range(jmax + 1):
                    r = j - 4 * qc
                    lo = max(r, 0) * P
                    for h in range(2):
                        s_ps = ps_scr.tile([P, QCH], F32, tag="scr", name="s_ps")
                        nc.tensor.matmul(
                            s_ps[:, lo:],
                            kt_sb[:, j * P : (j + 1) * P],
                            qt_sb[h][:, q0 + lo : q0 + QCH],
                            start=True, stop=True,
                        )
                        e = expp.tile([P, QCH], BF16, tag="e", name="e")
                        nc.scalar.activation(
                            e[:, lo:], s_ps[:, lo:], AF.Exp, scale=QK_SCALE
                        )
                        if r >= 0:
                            nc.vector.tensor_mul(
                                e[:, lo : lo + P], e[:, lo : lo + P], mask[:]
                            )
                        pend.append((j, h, e, lo))
                    while len(pend) > 2:
                        emit_pv(*pend.pop(0))
                while pend:
                    emit_pv(*pend.pop(0))
                return out_ps, den_t

            def attn_div(qc, out_ps, den_t):
                q_sl = slice(qc * QCH, (qc + 1) * QCH)
                for h in range(2):
                    rd = small.tile([1, QCH], F32, tag="rd", name="rd")
                    nc.vector.reciprocal(rd[:], den_t[32 * h : 32 * h + 1, :])
                    rdb = rdbp.tile([P, QCH], F32, tag="rdb", name="rdb")
                    nc.gpsimd.partition_broadcast(rdb[:], rd[:])
                    nc.vector.tensor_mul(attn_sb[h][:, q_sl], out_ps[h][:], rdb[:])

            def do_wo(qc):
                """Output projection for q chunk qc; one output DMA per chunk."""
                q_sl = slice(qc * QCH, (qc + 1) * QCH)
                ob = ostage.tile([P, KC, QCH], BF16, tag="ob", name="ob")
                for oc in range(KC):
                    o_ps = ps_main.tile([P, QCH], F32, tag="main", name="o_ps")
                    nc.tensor.matmul(
                        o_ps[:], wo_sb[:, 0, oc * P : (oc + 1) * P],
                        attn_sb[0][:, q_sl], start=True, stop=False,
                    )
                    nc.tensor.matmul(
                        o_ps[:], wo_sb[:, 1, oc * P : (oc + 1) * P],
                        attn_sb[1][:, q_sl], start=False, stop=True,
                    )
                    nc.vector.tensor_copy(ob[:, oc, :], o_ps[:])
                nc.scalar.dma_start(
                    out_d.ap()[:, :, q_sl].rearrange("o p s -> p o s"), ob[:]
                )

            # Phase-separated schedule: all projections, then attention.
            # Keeps the Activation engine on one table set per phase
            # (Square/Sqrt/Copy during projections, Exp during attention)
            # so only 2 table loads happen in the whole program.
            for rep in range(repeats):
                for p in range(NPASS):
                    proj_pass(p)
                for qc in range(NQC):
                    acc = attention(qc)
                    attn_div(qc, *acc)
                    if qc > 0:
                        do_wo(qc - 1)
                do_wo(NQC - 1)
                if dbg:
                    for h in range(2):
                        nc.sync.dma_start(dbg_qt[h].ap(), qt_sb[h][:])
                        nc.sync.dma_start(dbg_at[h].ap(), attn_sb[h][:])
                    nc.sync.dma_start(dbg_kt.ap(), kt_sb[:])
                    nc.sync.dma_start(dbg_v.ap(), v_sb[:])

    nc.compile()
    return nc


def _get_program(repeats=1):
    if repeats not in _PROGRAM:
        _PROGRAM[repeats] = _build_program(repeats)
    return _PROGRAM[repeats]


def _pack_x(a):
    """[S, HID] -> [NPASS, P, KT, 2, 256] with hid = kt*256 + i*128 + p."""
    return np.ascontiguousarray(
        a.reshape(NPASS, SCP * P, KT, 2, P).transpose(0, 4, 2, 3, 1)
    )


def _pack_w(a):
    """[HID, 512] -> [P, KT, 2, 512] with hid = kt*256 + i*128 + p."""
    return np.ascontiguousarray(a.reshape(KT, 2, P, 512).transpose(2, 0, 1, 3))


def _host_prepare(inputs):
    """Shard + lay out inputs for the 8 cores."""
    hs = np.asarray(inputs["hidden_states"], dtype=np.float32).reshape(S, HID)
    mu = np.asarray(inputs["mu_prev"], dtype=np.float32).reshape(S, HID)
    wq = np.asarray(inputs["wq"], dtype=np.float32)
    wk = np.asarray(inputs["wk"], dtype=np.float32)
    wv = np.asarray(inputs["wv"], dtype=np.float32)
    wo = np.asarray(inputs["wo"], dtype=np.float32)
    wmq = np.asarray(inputs["wmq"], dtype=np.float32)
    wmk = np.asarray(inputs["wmk"], dtype=np.float32)
    wmv = np.asarray(inputs["wmv"], dtype=np.float32)
    qw = np.asarray(inputs["q_norm_w"], dtype=np.float32)
    kw = np.asarray(inputs["k_norm_w"], dtype=np.float32)

    # hi/lo fp8 split of x; single fp8 for mu (its term is 10x smaller)
    xh8 = hs.astype(NP_E4)
    xl8 = (hs - xh8.astype(np.float32)).astype(NP_E4)
    mu8 = mu.astype(NP_E4)
    xh_p = _pack_x(xh8)
    xl_p = _pack_x(xl8)
    mu_p = _pack_x(mu8)

    # RoPE tables in [s, d] layout with rotate-half sign and norm weight baked in
    inv = 1.0 / (ROPE_THETA ** (np.arange(0, HEAD_DIM, 2, dtype=np.float32) / HEAD_DIM))
    ang = np.arange(S, dtype=np.float32)[:, None] * inv[None, :]  # [S, 64]
    emb = np.concatenate([ang, ang], axis=-1)  # [S, 128]
    cos_e = np.cos(emb)
    sin_e = np.sin(emb)
    sin_s = np.concatenate([-sin_e[:, :64], sin_e[:, 64:]], axis=-1)

    def tables(w):
        w_shift = np.concatenate([w[64:], w[:64]])
        cos_t = (cos_e * w[None, :]).astype(np.float32).reshape(SC, P, HEAD_DIM)
        sin_t = (sin_s * w_shift[None, :]).astype(np.float32).reshape(SC, P, HEAD_DIM)
        return np.ascontiguousarray(cos_t), np.ascontiguousarray(sin_t)

    cq, sq = tables(qw)
    ck, sk = tables(kw)

    in_maps = []
    for c in range(N_CORES):
        g = c // 2
        wq_s = wq[256 * c : 256 * (c + 1)]      # [256, HID]
        wmq_s = wmq[256 * c : 256 * (c + 1)]
        wk_s = wk[P * g : P * (g + 1)]          # [128, HID]
        wmk_s = wmk[P * g : P * (g + 1)]
        wv_s = wv[P * g : P * (g + 1)]
        wmv_s = wmv[P * g : P * (g + 1)]
        w_all = np.concatenate([wq_s.T, wk_s.T, wv_s.T], axis=1) * WSCALE  # [HID, 512]
        wm_all = np.concatenate([wmq_s.T, wmk_s.T, wmv_s.T], axis=1) * WSCALE
        wh8 = w_all.astype(NP_E4)
        wl8 = (w_all - wh8.astype(np.float32)).astype(NP_E4)
        wm8 = wm_all.astype(NP_E4)
        woT_c = wo[:, 256 * c : 256 * (c + 1)].T                     # [256, HID]
        in_maps.append(
            {
                "xh": xh_p,
                "xl": xl_p,
                "mu8": mu_p,
                "wh": _pack_w(wh8),
                "wl": _pack_w(wl8),
                "wm8": _pack_w(wm8),
                "woT": np.ascontiguousarray(woT_c).astype(NP_BF16).reshape(2, P, HID),
                "cosq": cq,
                "sinq": sq,
                "cosk": ck,
                "sink": sk,
            }
        )
    return in_maps


def run(inputs, trace=False):
    """Run the SPMD kernel; returns (full_output, exec_time_ns_or_None)."""
    nc = _get_program()
    in_maps = _host_prepare(inputs)
    res = run_bass_kernel_spmd(
        nc, in_maps, core_ids=list(range(N_CORES)), trace=trace
    )
    total = np.zeros((HID, S), dtype=np.float32)
    for c in range(N_CORES):
        total += res.results[c]["out"].astype(np.float32).reshape(HID, S)
    out = np.ascontiguousarray(total.T).reshape(B, S, HID).astype(np.float32)
    return out, res.exec_time_ns


def kernel(**inputs) -> np.ndarray:
    out, _ = run(inputs, trace=False)
    return out


# revision 24
# speedup vs baseline: 1.6101x; 1.0631x over previous
"""Trainium2 Bass kernel for nn_ComplexityAttention (GQA attention block).

Computation (B=1, S=2048, HID=2048, 16 Q heads / 4 KV heads, D=128):
  q/k/v = x @ W^T + mu @ Wm^T           (fused mu-guided projections)
  per-head RMSNorm on q, k; RoPE; causal GQA attention; out @ wo^T.

Sharding: tensor-parallel over heads across 8 NeuronCores. Core c owns
Q heads {2c, 2c+1} and KV head c//2 (KV work duplicated per core pair).
Each core produces a partial output (its heads' slice of wo applied),
host sums the 8 partials.

Device-side strategy (v2, fp8-DoubleRow projections):
  - Projections run on the PE in fp8e4m3 DoubleRow mode (256-deep
    contraction per matmul, 0.5 cycles/row): x is split hi+lo
    (x ~= xh + xl, both e4m3) and W is pre-scaled by 64 and split
    hi+lo; x@W ~= xh@Wh + xl@Wh + xh@Wl.  mu@Wm uses a single fp8
    term (the mu path is 10x smaller, quantization is negligible).
    The 64x weight scale cancels inside RMSNorm for q/k and is divided
    out during the V copy.
  - rstd = exp(-0.5*ln(var+eps)) so the Activation engine only ever
    uses {Square, Ln, Exp, Copy} - one table set, no table reloads.
  - Scores computed transposed S^T[kv, q] = K^T.T @ Q^T, softmax
    without max-subtraction, denominator via ones-vector matmul.
    Causal-diagonal blocks narrowed to the live q-range (no wasted
    columns); score matmuls emitted one kv-block ahead of the PV
    matmuls so the PE never waits on the Exp latency.
  - Output projection per 512-q chunk, staged to bf16 and written as
    one DMA per chunk; all inputs arrive as a handful of large DMAs
    (per-partition-contiguous layouts prepared on host).
"""

import sys

for _p in ("/opt/trn_rl_repo", "/root/.axon_site/_ro/trn_rl_repo"):
    if _p not in sys.path:
        sys.path.insert(0, _p)

import numpy as np
import ml_dtypes

import concourse.bass as bass
import concourse.bacc as bacc
import concourse.mybir as mybir
import concourse.tile as tile
from concourse.bass_utils import run_bass_kernel_spmd
from concourse.masks import make_identity

# Problem constants (hardcoded per contract)
B, S, HID = 1, 2048, 2048
NUM_HEADS, NUM_KV_HEADS, HEAD_DIM = 16, 4, 128
ROPE_THETA = 10000.0
EPS = 1e-6
N_CORES = 8

P = 128
KC = HID // P            # 16 contraction chunks of 128
KT = 8                   # contraction chunk-pairs (256 wide) for DoubleRow
SC = S // P              # 16 sequence chunks of 128
SCP = 2                  # s-chunks per projection pass
NPASS = SC // SCP        # 8 projection passes of 256 seq positions
QCH = 512                # attention q-chunk (one PSUM bank)
NQC = S // QCH           # 4
WSCALE = 64.0            # weight pre-scale for fp8 (cancels in norm / V copy)
QK_SCALE = 1.0 / float(np.sqrt(HEAD_DIM))

BF16 = mybir.dt.bfloat16
F32 = mybir.dt.float32
FP8 = mybir.dt.float8e4
NP_BF16 = ml_dtypes.bfloat16
NP_E4 = ml_dtypes.float8_e4m3

_PROGRAM = {}


def _build_program(repeats=1, dbg=False):
    """Build the per-core Bass/Tile program (identical on all 8 cores)."""
    AF = mybir.ActivationFunctionType
    OP = mybir.AluOpType
    DR = mybir.MatmulPerfMode.DoubleRow

    nc = bacc.Bacc(trn_type="TRN2", debug=False)

    # ---- DRAM I/O ----
    # x/mu in fp8, per-partition contiguous: [pass][p][kt][2][256]
    xh_d = nc.dram_tensor("xh", [NPASS, P, KT, 2, SCP * P], FP8, kind="ExternalInput")
    xl_d = nc.dram_tensor("xl", [NPASS, P, KT, 2, SCP * P], FP8, kind="ExternalInput")
    mu_d = nc.dram_tensor("mu8", [NPASS, P, KT, 2, SCP * P], FP8, kind="ExternalInput")
    # packed projection weights (64x pre-scaled): [p][kt][2][512] (q0|q1|k|v)
    wh_d = nc.dram_tensor("wh", [P, KT, 2, 512], FP8, kind="ExternalInput")
    wl_d = nc.dram_tensor("wl", [P, KT, 2, 512], FP8, kind="ExternalInput")
    wm_d = nc.dram_tensor("wm8", [P, KT, 2, 512], FP8, kind="ExternalInput")
    woT = nc.dram_tensor("woT", [2, P, HID], BF16, kind="ExternalInput")
    cosq = nc.dram_tensor("cosq", [SC, P, HEAD_DIM], F32, kind="ExternalInput")
    sinq = nc.dram_tensor("sinq", [SC, P, HEAD_DIM], F32, kind="ExternalInput")
    cosk = nc.dram_tensor("cosk", [SC, P, HEAD_DIM], F32, kind="ExternalInput")
    sink = nc.dram_tensor("sink", [SC, P, HEAD_DIM], F32, kind="ExternalInput")
    out_d = nc.dram_tensor("out", [KC, P, S], BF16, kind="ExternalOutput")
    if dbg:
        dbg_qt = [nc.dram_tensor(f"dbg_qt{h}", [P, S], BF16, kind="ExternalOutput")
                  for h in range(2)]
        dbg_kt = nc.dram_tensor("dbg_kt", [P, S], BF16, kind="ExternalOutput")
        dbg_v = nc.dram_tensor("dbg_v", [P, SC, HEAD_DIM], BF16, kind="ExternalOutput")
        dbg_at = [nc.dram_tensor(f"dbg_at{h}", [P, S], BF16, kind="ExternalOutput")
                  for h in range(2)]

    with tile.TileContext(nc) as tc:
        with (
            tc.tile_pool(name="persist", bufs=1) as persist,
            tc.tile_pool(name="stream", bufs=4) as stream,
            tc.tile_pool(name="tmp", bufs=6) as tmp,
            tc.tile_pool(name="small", bufs=8) as small,
            tc.tile_pool(name="expp", bufs=6) as expp,
            tc.tile_pool(name="rdbp", bufs=2) as rdbp,
            tc.tile_pool(name="ostage", bufs=2) as ostage,
            tc.tile_pool(name="ps_main", bufs=3, space="PSUM") as ps_main,
            tc.tile_pool(name="ps_scr", bufs=2, space="PSUM") as ps_scr,
            tc.tile_pool(name="ps_out", bufs=2, space="PSUM") as ps_out,
            tc.tile_pool(name="ps_den", bufs=1, space="PSUM") as ps_den,
        ):
            # ---- persistent SBUF tensors ----
            wh_sb = persist.tile([P, KT, 2, 512], FP8, name="wh_sb")
            wl_sb = persist.tile([P, KT, 2, 512], FP8, name="wl_sb")
            wm_sb = persist.tile([P, KT, 2, 512], FP8, name="wm_sb")
            wo_sb = persist.tile([P, 2, HID], BF16, name="wo_sb")
            cq_sb = persist.tile([P, SC, HEAD_DIM], F32, name="cq_sb")
            sq_sb = persist.tile([P, SC, HEAD_DIM], F32, name="sq_sb")
            ck_sb = persist.tile([P, SC, HEAD_DIM], F32, name="ck_sb")
            sk_sb = persist.tile([P, SC, HEAD_DIM], F32, name="sk_sb")
            qt_sb = [persist.tile([P, S], BF16, name=f"qt{h}_sb") for h in range(2)]
            kt_sb = persist.tile([P, S], BF16, name="kt_sb")
            v_sb = persist.tile([P, SC, HEAD_DIM], BF16, name="v_sb")
            attn_sb = [persist.tile([P, S], BF16, name=f"attn{c}_sb") for c in range(2)]
            ident = persist.tile([P, P], BF16, name="ident")
            ones_sb = persist.tile([P, 1], BF16, name="ones_sb")
            eps_sb = persist.tile([P, 1], F32, name="eps_sb")
            mask = persist.tile([P, P], BF16, name="mask")

            make_identity(nc, ident[:])
            nc.gpsimd.memset(ones_sb[:], 1.0)
            # bias for Sqrt: var/HEAD_DIM + WSCALE^2*eps = WSCALE^2*(mean+eps)
            nc.gpsimd.memset(eps_sb[:], EPS * WSCALE * WSCALE)
            # causal mask for 128x128 diagonal blocks (scores transposed:
            # rows=kv, cols=q; keep where q >= kv)
            nc.gpsimd.memset(mask[:], 1.0)
            nc.gpsimd.affine_select(
                out=mask[:],
                in_=mask[:],
                compare_op=mybir.AluOpType.is_ge,
                fill=0.0,
                base=0,
                pattern=[[1, P]],
                channel_multiplier=-1,
            )

            # ---- weight DMAs (x-pass DMAs are interleaved by proj_pass;
            # rope tables and wo are emitted later so the first projection
            # passes aren't starved behind them on the DMA bus) ----
            nc.scalar.dma_start(wh_sb[:], wh_d.ap())
            nc.scalar.dma_start(wl_sb[:], wl_d.ap())
            nc.scalar.dma_start(wm_sb[:], wm_d.ap())

            # (hidx within packed 512 cols, cos table, sin table, dest)
            norm_specs = [
                (2, ck_sb, sk_sb, kt_sb),
                (0, cq_sb, sq_sb, qt_sb[0]),
                (1, cq_sb, sq_sb, qt_sb[1]),
            ]

            def pass_dma(p):
                """Prefetch the x/mu tiles for projection pass p."""
                xh_t = stream.tile([P, KT, 2, SCP * P], FP8, tag="xh", name="xh_t")
                xl_t = stream.tile([P, KT, 2, SCP * P], FP8, tag="xl", name="xl_t")
                mu_t = stream.tile([P, KT, 2, SCP * P], FP8, tag="mu", name="mu_t")
                nc.sync.dma_start(xh_t[:], xh_d.ap()[p])
                nc.sync.dma_start(xl_t[:], xl_d.ap()[p])
                nc.sync.dma_start(mu_t[:], mu_d.ap()[p])
                return xh_t, xl_t, mu_t

            def proj_pass(p, tiles):
                """Project s-columns [256p, 256p+256): fp8 DoubleRow matmuls,
                then RMSNorm+RoPE+transpose for q0/q1/k, V copy."""
                xh_t, xl_t, mu_t = tiles
                pp = [
                    ps_main.tile([P, 512], F32, tag="main", name=f"pp{i}")
                    for i in range(SCP)
                ]
                for i2 in range(SCP):
                    sl = slice(i2 * P, (i2 + 1) * P)
                    for kt in range(KT):
                        nc.tensor.matmul(
                            pp[i2][:], xh_t[:, kt, :, sl], wh_sb[:, kt],
                            start=(kt == 0), stop=False, perf_mode=DR,
                        )
                        nc.tensor.matmul(
                            pp[i2][:], xl_t[:, kt, :, sl], wh_sb[:, kt],
                            start=False, stop=False, perf_mode=DR,
                        )
                        nc.tensor.matmul(
                            pp[i2][:], xh_t[:, kt, :, sl], wl_sb[:, kt],
                            start=False, stop=False, perf_mode=DR,
                        )
                        nc.tensor.matmul(
                            pp[i2][:], mu_t[:, kt, :, sl], wm_sb[:, kt],
                            start=False, stop=(kt == KT - 1), perf_mode=DR,
                        )
                # normalize + rope + transpose into [d, s] layout
                for i2 in range(SCP):
                    sc = p * SCP + i2
                    ps = pp[i2]
                    for hidx, c_sb, s_sb, dst in norm_specs:
                        off = hidx * P
                        sqv = tmp.tile([P, HEAD_DIM], F32, tag="sqv", name="sqv")
                        var = small.tile([P, 1], F32, tag="var", name="var")
                        nc.scalar.activation(
                            sqv[:], ps[:, off : off + P], AF.Square, accum_out=var[:]
                        )
                        # std64 = sqrt(var/HEAD_DIM + WSCALE^2*eps)
                        #       = WSCALE*sqrt(mean(q^2)+eps); its reciprocal
                        # also cancels the weight pre-scale still in psum
                        std = small.tile([P, 1], F32, tag="std", name="std")
                        nc.scalar.activation(
                            std[:], var[:], AF.Sqrt,
                            scale=1.0 / HEAD_DIM, bias=eps_sb[:],
                        )
                        rstd = small.tile([P, 1], F32, tag="rstd", name="rstd")
                        nc.vector.reciprocal(rstd[:], std[:])
                        t1 = tmp.tile([P, HEAD_DIM], F32, tag="t1", name="t1")
                        nc.vector.scalar_tensor_tensor(
                            t1[:], ps[:, off : off + P], rstd[:], c_sb[:, sc, :],
                            op0=OP.mult, op1=OP.mult,
                        )
                        t2 = tmp.tile([P, HEAD_DIM], F32, tag="t2", name="t2")
                        nc.vector.scalar_tensor_tensor(
                            t2[:, 0:64], ps[:, off + 64 : off + P], rstd[:],
                            s_sb[:, sc, 0:64], op0=OP.mult, op1=OP.mult,
                        )
                        nc.vector.scalar_tensor_tensor(
                            t2[:, 64:P], ps[:, off : off + 64], rstd[:],
                            s_sb[:, sc, 64:P], op0=OP.mult, op1=OP.mult,
                        )
                        qsd = tmp.tile([P, HEAD_DIM], BF16, tag="qsd", name="qsd")
                        nc.vector.tensor_add(qsd[:], t1[:], t2[:])
                        tr = ps_scr.tile([P, P], BF16, tag="scr", name="tr")
                        nc.tensor.transpose(tr[:], qsd[:], ident[:])
                        nc.vector.tensor_copy(dst[:, sc * P : (sc + 1) * P], tr[:])
                    # V: copy (cast) with 1/WSCALE to undo the weight pre-scale
                    nc.scalar.activation(
                        v_sb[:, sc, :], ps[:, 384:512], AF.Copy, scale=1.0 / WSCALE
                    )

            def attention(qc, filler=None):
                """Scores/exp/PV/den for q-chunk qc (columns [512qc,512qc+512)).
                Causal-narrowed; PV lags scores by one kv-block so the PE
                never waits on Exp.  `filler` emits independent PE work (the
                previous chunk's wo matmuls) between kv-blocks to absorb
                latency bubbles."""
                jmax = 4 * qc + 3
                q0 = qc * QCH
                out_ps = [
                    ps_out.tile([P, QCH], F32, tag="out", name=f"out_ps{h}")
                    for h in range(2)
                ]
                den_t = ps_den.tile([64, QCH], F32, tag="den", name="den_t")
                pend = []

                def emit_pv(j, h, e, lo):
                    nc.tensor.matmul(
                        out_ps[h][:, lo:], v_sb[:, j, :], e[:, lo:],
                        start=(j == 0), stop=(j == jmax), skip_group_check=True,
                    )
                    nc.tensor.matmul(
                        den_t[32 * h : 32 * h + 1, lo:], ones_sb[:], e[:, lo:],
                        start=(j == 0), stop=(j == jmax), skip_group_check=True,
                    )

                for j in range(jmax + 1):
                    r = j - 4 * qc
                    lo = max(r, 0) * P
                    for h in range(2):
                        s_ps = ps_scr.tile([P, QCH], F32, tag="scr", name="s_ps")
                        nc.tensor.matmul(
                            s_ps[:, lo:],
                            kt_sb[:, j * P : (j + 1) * P],
                            qt_sb[h][:, q0 + lo : q0 + QCH],
                            start=True, stop=True,
                        )
                        e = expp.tile([P, QCH], BF16, tag="e", name="e")
                        nc.scalar.activation(
                            e[:, lo:], s_ps[:, lo:], AF.Exp, scale=QK_SCALE
                        )
                        if r >= 0:
                            nc.vector.tensor_mul(
                                e[:, lo : lo + P], e[:, lo : lo + P], mask[:]
                            )
                        pend.append((j, h, e, lo))
                    while len(pend) > 2:
                        emit_pv(*pend.pop(0))
                    if filler is not None:
                        next(filler, None)
                while pend:
                    emit_pv(*pend.pop(0))
                return out_ps, den_t

            def attn_div(qc, out_ps, den_t):
                q_sl = slice(qc * QCH, (qc + 1) * QCH)
                for h in range(2):
                    rd = small.tile([1, QCH], F32, tag="rd", name="rd")
                    nc.vector.reciprocal(rd[:], den_t[32 * h : 32 * h + 1, :])
                    rdb = rdbp.tile([P, QCH], F32, tag="rdb", name="rdb")
                    nc.gpsimd.partition_broadcast(rdb[:], rd[:])
                    nc.vector.tensor_mul(attn_sb[h][:, q_sl], out_ps[h][:], rdb[:])

            def do_wo(qc):
                """Output projection for q chunk qc, as a generator yielding
                after every 2 output chunks; output DMA per 4 chunks."""
                q_sl = slice(qc * QCH, (qc + 1) * QCH)
                ob = ostage.tile([P, KC, QCH], BF16, tag="ob", name="ob")
                for oc in range(KC):
                    o_ps = ps_main.tile([P, QCH], F32, tag="main", name="o_ps")
                    nc.tensor.matmul(
                        o_ps[:], wo_sb[:, 0, oc * P : (oc + 1) * P],
                        attn_sb[0][:, q_sl], start=True, stop=False,
                    )
                    nc.tensor.matmul(
                        o_ps[:], wo_sb[:, 1, oc * P : (oc + 1) * P],
                        attn_sb[1][:, q_sl], start=False, stop=True,
                    )
                    nc.vector.tensor_copy(ob[:, oc, :], o_ps[:])
                    if oc % 4 == 3:
                        g = oc - 3
                        nc.scalar.dma_start(
                            out_d.ap()[g : g + 4, :, q_sl].rearrange(
                                "o p s -> p o s"
                            ),
                            ob[:, g : g + 4, :],
                        )
                    if oc % 2 == 1:
                        yield

            def drain(gen):
                if gen is not None:
                    for _ in gen:
                        pass

            # Phase-separated schedule: all projections, then attention.
            # Keeps the Activation engine on one table set per phase
            # (Square/Sqrt/Copy during projections, Exp during attention)
            # so only 2 table loads happen in the whole program.  The rope
            # tables and wo weights are DMA'd mid-stream, after the early
            # x-tiles they'd otherwise delay.
            for rep in range(repeats):
                tiles = {0: pass_dma(0), 1: pass_dma(1)}
                if rep == 0:
                    # rope tables in small chunks so the DMA-bus FIFO
                    # interleaves them fairly with the x-tile stream
                    for g in range(0, SC, 4):
                        for dst, src in (
                            (ck_sb, cosk), (sk_sb, sink),
                            (cq_sb, cosq), (sq_sb, sinq),
                        ):
                            nc.scalar.dma_start(
                                dst[:, g : g + 4, :],
                                src.ap()[g : g + 4].rearrange("s p d -> p s d"),
                            )
                for p in range(NPASS):
                    if p + 2 < NPASS:
                        tiles[p + 2] = pass_dma(p + 2)
                    proj_pass(p, tiles.pop(p))
                    if p == 4 and rep == 0:
                        nc.scalar.dma_start(
                            wo_sb[:], woT.ap().rearrange("c p o -> p c o"))
                wo_gen = None
                for qc in range(NQC):
                    acc = attention(qc, filler=wo_gen)
                    drain(wo_gen)
                    attn_div(qc, *acc)
                    wo_gen = do_wo(qc)
                drain(wo_gen)
                if dbg:
                    for h in range(2):
                        nc.sync.dma_start(dbg_qt[h].ap(), qt_sb[h][:])
                        nc.sync.dma_start(dbg_at[h].ap(), attn_sb[h][:])
                    nc.sync.dma_start(dbg_kt.ap(), kt_sb[:])
                    nc.sync.dma_start(dbg_v.ap(), v_sb[:])

    nc.compile()
    return nc


def _get_program(repeats=1):
    if repeats not in _PROGRAM:
        _PROGRAM[repeats] = _build_program(repeats)
    return _PROGRAM[repeats]


def _pack_x(a):
    """[S, HID] -> [NPASS, P, KT, 2, 256] with hid = kt*256 + i*128 + p."""
    return np.ascontiguousarray(
        a.reshape(NPASS, SCP * P, KT, 2, P).transpose(0, 4, 2, 3, 1)
    )


def _pack_w(a):
    """[HID, 512] -> [P, KT, 2, 512] with hid = kt*256 + i*128 + p."""
    return np.ascontiguousarray(a.reshape(KT, 2, P, 512).transpose(2, 0, 1, 3))


def _host_prepare(inputs):
    """Shard + lay out inputs for the 8 cores."""
    hs = np.asarray(inputs["hidden_states"], dtype=np.float32).reshape(S, HID)
    mu = np.asarray(inputs["mu_prev"], dtype=np.float32).reshape(S, HID)
    wq = np.asarray(inputs["wq"], dtype=np.float32)
    wk = np.asarray(inputs["wk"], dtype=np.float32)
    wv = np.asarray(inputs["wv"], dtype=np.float32)
    wo = np.asarray(inputs["wo"], dtype=np.float32)
    wmq = np.asarray(inputs["wmq"], dtype=np.float32)
    wmk = np.asarray(inputs["wmk"], dtype=np.float32)
    wmv = np.asarray(inputs["wmv"], dtype=np.float32)
    qw = np.asarray(inputs["q_norm_w"], dtype=np.float32)
    kw = np.asarray(inputs["k_norm_w"], dtype=np.float32)

    # hi/lo fp8 split of x; single fp8 for mu (its term is 10x smaller)
    xh8 = hs.astype(NP_E4)
    xl8 = (hs - xh8.astype(np.float32)).astype(NP_E4)
    mu8 = mu.astype(NP_E4)
    xh_p = _pack_x(xh8)
    xl_p = _pack_x(xl8)
    mu_p = _pack_x(mu8)

    # RoPE tables in [s, d] layout with rotate-half sign and norm weight baked in
    inv = 1.0 / (ROPE_THETA ** (np.arange(0, HEAD_DIM, 2, dtype=np.float32) / HEAD_DIM))
    ang = np.arange(S, dtype=np.float32)[:, None] * inv[None, :]  # [S, 64]
    emb = np.concatenate([ang, ang], axis=-1)  # [S, 128]
    cos_e = np.cos(emb)
    sin_e = np.sin(emb)
    sin_s = np.concatenate([-sin_e[:, :64], sin_e[:, 64:]], axis=-1)

    def tables(w):
        w_shift = np.concatenate([w[64:], w[:64]])
        cos_t = (cos_e * w[None, :]).astype(np.float32).reshape(SC, P, HEAD_DIM)
        sin_t = (sin_s * w_shift[None, :]).astype(np.float32).reshape(SC, P, HEAD_DIM)
        return np.ascontiguousarray(cos_t), np.ascontiguousarray(sin_t)

    cq, sq = tables(qw)
    ck, sk = tables(kw)

    in_maps = []
    for c in range(N_CORES):
        g = c // 2
        wq_s = wq[256 * c : 256 * (c + 1)]      # [256, HID]
        wmq_s = wmq[256 * c : 256 * (c + 1)]
        wk_s = wk[P * g : P * (g + 1)]          # [128, HID]
        wmk_s = wmk[P * g : P * (g + 1)]
        wv_s = wv[P * g : P * (g + 1)]
        wmv_s = wmv[P * g : P * (g + 1)]
        w_all = np.concatenate([wq_s.T, wk_s.T, wv_s.T], axis=1) * WSCALE  # [HID, 512]
        wm_all = np.concatenate([wmq_s.T, wmk_s.T, wmv_s.T], axis=1) * WSCALE
        wh8 = w_all.astype(NP_E4)
        wl8 = (w_all - wh8.astype(np.float32)).astype(NP_E4)
        wm8 = wm_all.astype(NP_E4)
        woT_c = wo[:, 256 * c : 256 * (c + 1)].T                     # [256, HID]
        in_maps.append(
            {
                "xh": xh_p,
                "xl": xl_p,
                "mu8": mu_p,
                "wh": _pack_w(wh8),
                "wl": _pack_w(wl8),
                "wm8": _pack_w(wm8),
                "woT": np.ascontiguousarray(woT_c).astype(NP_BF16).reshape(2, P, HID),
                "cosq": cq,
                "sinq": sq,
                "cosk": ck,
                "sink": sk,
            }
        )
    return in_maps


def run(inputs, trace=False):
    """Run the SPMD kernel; returns (full_output, exec_time_ns_or_None)."""
    nc = _get_program()
    in_maps = _host_prepare(inputs)
    res = run_bass_kernel_spmd(
        nc, in_maps, core_ids=list(range(N_CORES)), trace=trace
    )
    total = np.zeros((HID, S), dtype=np.float32)
    for c in range(N_CORES):
        total += res.results[c]["out"].astype(np.float32).reshape(HID, S)
    out = np.ascontiguousarray(total.T).reshape(B, S, HID).astype(np.float32)
    return out, res.exec_time_ns


def kernel(**inputs) -> np.ndarray:
    out, _ = run(inputs, trace=False)
    return out


# revision 34
# speedup vs baseline: 1.6147x; 1.0028x over previous
"""Trainium2 Bass kernel for nn_ComplexityAttention (GQA attention block).

Computation (B=1, S=2048, HID=2048, 16 Q heads / 4 KV heads, D=128):
  q/k/v = x @ W^T + mu @ Wm^T           (fused mu-guided projections)
  per-head RMSNorm on q, k; RoPE; causal GQA attention; out @ wo^T.

Sharding: tensor-parallel over heads across 8 NeuronCores. Core c owns
Q heads {2c, 2c+1} and KV head c//2 (KV work duplicated per core pair).
Each core produces a partial output (its heads' slice of wo applied),
host sums the 8 partials.

Device-side strategy (v2, fp8-DoubleRow projections):
  - Projections run on the PE in fp8e4m3 DoubleRow mode (256-deep
    contraction per matmul, 0.5 cycles/row): x is split hi+lo
    (x ~= xh + xl, both e4m3) and W is pre-scaled by 64 and split
    hi+lo; x@W ~= xh@Wh + xl@Wh + xh@Wl.  mu@Wm uses a single fp8
    term (the mu path is 10x smaller, quantization is negligible).
    The 64x weight scale cancels inside RMSNorm for q/k and is divided
    out during the V copy.
  - rstd = exp(-0.5*ln(var+eps)) so the Activation engine only ever
    uses {Square, Ln, Exp, Copy} - one table set, no table reloads.
  - Scores computed transposed S^T[kv, q] = K^T.T @ Q^T, softmax
    without max-subtraction, denominator via ones-vector matmul.
    Causal-diagonal blocks narrowed to the live q-range (no wasted
    columns); score matmuls emitted one kv-block ahead of the PV
    matmuls so the PE never waits on the Exp latency.
  - Output projection per 512-q chunk, staged to bf16 and written as
    one DMA per chunk; all inputs arrive as a handful of large DMAs
    (per-partition-contiguous layouts prepared on host).
"""

import sys

for _p in ("/opt/trn_rl_repo", "/root/.axon_site/_ro/trn_rl_repo"):
    if _p not in sys.path:
        sys.path.insert(0, _p)

import numpy as np
import ml_dtypes

import concourse.bass as bass
import concourse.bacc as bacc
import concourse.mybir as mybir
import concourse.tile as tile
from concourse.bass_utils import run_bass_kernel_spmd
from concourse.masks import make_identity

# Problem constants (hardcoded per contract)
B, S, HID = 1, 2048, 2048
NUM_HEADS, NUM_KV_HEADS, HEAD_DIM = 16, 4, 128
ROPE_THETA = 10000.0
EPS = 1e-6
N_CORES = 8

P = 128
KC = HID // P            # 16 contraction chunks of 128
KT = 8                   # contraction chunk-pairs (256 wide) for DoubleRow
SC = S // P              # 16 sequence chunks of 128
SCP = 2                  # s-chunks per projection pass
NPASS = SC // SCP        # 8 projection passes of 256 seq positions
QCH = 512                # attention q-chunk (one PSUM bank)
NQC = S // QCH           # 4
WSCALE = 64.0            # weight pre-scale for fp8 (cancels in norm / V copy)
QK_SCALE = 1.0 / float(np.sqrt(HEAD_DIM))

BF16 = mybir.dt.bfloat16
F32 = mybir.dt.float32
FP8 = mybir.dt.float8e4
NP_BF16 = ml_dtypes.bfloat16
NP_E4 = ml_dtypes.float8_e4m3

_PROGRAM = {}


def _build_program(repeats=1, dbg=False, shared_rope=True):
    """Build the per-core Bass/Tile program (identical on all 8 cores)."""
    AF = mybir.ActivationFunctionType
    OP = mybir.AluOpType
    DR = mybir.MatmulPerfMode.DoubleRow

    nc = bacc.Bacc(trn_type="TRN2", debug=False)

    # ---- DRAM I/O ----
    # x/mu in fp8, per-partition contiguous: [pass][p][kt][2][256]
    xh_d = nc.dram_tensor("xh", [NPASS, P, KT, 2, SCP * P], FP8, kind="ExternalInput")
    xl_d = nc.dram_tensor("xl", [NPASS, P, KT, 2, SCP * P], FP8, kind="ExternalInput")
    mu_d = nc.dram_tensor("mu8", [NPASS, P, KT, 2, SCP * P], FP8, kind="ExternalInput")
    # packed projection weights (64x pre-scaled): [p][kt][2][512] (q0|q1|k|v)
    wh_d = nc.dram_tensor("wh", [P, KT, 2, 512], FP8, kind="ExternalInput")
    wl_d = nc.dram_tensor("wl", [P, KT, 2, 512], FP8, kind="ExternalInput")
    wm_d = nc.dram_tensor("wm8", [P, KT, 2, 512], FP8, kind="ExternalInput")
    woT = nc.dram_tensor("woT", [2, P, HID], BF16, kind="ExternalInput")
    cosq = nc.dram_tensor("cosq", [SC, P, HEAD_DIM], F32, kind="ExternalInput")
    sinq = nc.dram_tensor("sinq", [SC, P, HEAD_DIM], F32, kind="ExternalInput")
    if not shared_rope:
        cosk = nc.dram_tensor("cosk", [SC, P, HEAD_DIM], F32, kind="ExternalInput")
        sink = nc.dram_tensor("sink", [SC, P, HEAD_DIM], F32, kind="ExternalInput")
    out_d = nc.dram_tensor("out", [KC, P, S], BF16, kind="ExternalOutput")
    if dbg:
        dbg_qt = [nc.dram_tensor(f"dbg_qt{h}", [P, S], BF16, kind="ExternalOutput")
                  for h in range(2)]
        dbg_kt = nc.dram_tensor("dbg_kt", [P, S], BF16, kind="ExternalOutput")
        dbg_v = nc.dram_tensor("dbg_v", [P, SC, HEAD_DIM], BF16, kind="ExternalOutput")
        dbg_at = [nc.dram_tensor(f"dbg_at{h}", [P, S], BF16, kind="ExternalOutput")
                  for h in range(2)]

    with tile.TileContext(nc) as tc:
        with (
            tc.tile_pool(name="persist", bufs=1) as persist,
            tc.tile_pool(name="stream", bufs=4) as stream,
            tc.tile_pool(name="tmp", bufs=6) as tmp,
            tc.tile_pool(name="small", bufs=8) as small,
            tc.tile_pool(name="expp", bufs=6) as expp,
            tc.tile_pool(name="rdbp", bufs=2) as rdbp,
            tc.tile_pool(name="ostage", bufs=2) as ostage,
            tc.tile_pool(name="ps_main", bufs=3, space="PSUM") as ps_main,
            tc.tile_pool(name="ps_scr", bufs=2, space="PSUM") as ps_scr,
            tc.tile_pool(name="ps_out", bufs=2, space="PSUM") as ps_out,
            tc.tile_pool(name="ps_den", bufs=1, space="PSUM") as ps_den,
        ):
            # ---- persistent SBUF tensors ----
            wh_sb = persist.tile([P, KT, 2, 512], FP8, name="wh_sb")
            wl_sb = persist.tile([P, KT, 2, 512], FP8, name="wl_sb")
            wm_sb = persist.tile([P, KT, 2, 512], FP8, name="wm_sb")
            wo_sb = persist.tile([P, 2, HID], BF16, name="wo_sb")
            cq_sb = persist.tile([P, SC, HEAD_DIM], F32, name="cq_sb")
            sq_sb = persist.tile([P, SC, HEAD_DIM], F32, name="sq_sb")
            if shared_rope:
                ck_sb, sk_sb = cq_sb, sq_sb
            else:
                ck_sb = persist.tile([P, SC, HEAD_DIM], F32, name="ck_sb")
                sk_sb = persist.tile([P, SC, HEAD_DIM], F32, name="sk_sb")
            qt_sb = [persist.tile([P, S], BF16, name=f"qt{h}_sb") for h in range(2)]
            kt_sb = persist.tile([P, S], BF16, name="kt_sb")
            v_sb = persist.tile([P, SC, HEAD_DIM], BF16, name="v_sb")
            attn_sb = [persist.tile([P, S], BF16, name=f"attn{c}_sb") for c in range(2)]
            ident = persist.tile([P, P], BF16, name="ident")
            ones_sb = persist.tile([P, 1], BF16, name="ones_sb")
            eps_sb = persist.tile([P, 1], F32, name="eps_sb")
            mask = persist.tile([P, P], BF16, name="mask")

            make_identity(nc, ident[:])
            nc.gpsimd.memset(ones_sb[:], 1.0)
            # bias for Sqrt: var/HEAD_DIM + WSCALE^2*eps = WSCALE^2*(mean+eps)
            nc.gpsimd.memset(eps_sb[:], EPS * WSCALE * WSCALE)
            # causal mask for 128x128 diagonal blocks (scores transposed:
            # rows=kv, cols=q; keep where q >= kv)
            nc.gpsimd.memset(mask[:], 1.0)
            nc.gpsimd.affine_select(
                out=mask[:],
                in_=mask[:],
                compare_op=mybir.AluOpType.is_ge,
                fill=0.0,
                base=0,
                pattern=[[1, P]],
                channel_multiplier=-1,
            )

            # ---- weight DMAs (x-pass DMAs are interleaved by proj_pass;
            # rope tables and wo are emitted later so the first projection
            # passes aren't starved behind them on the DMA bus) ----
            for half in range(2):
                kt_sl = slice(half * (KT // 2), (half + 1) * (KT // 2))
                nc.scalar.dma_start(wh_sb[:, kt_sl], wh_d.ap()[:, kt_sl])
                nc.scalar.dma_start(wl_sb[:, kt_sl], wl_d.ap()[:, kt_sl])
                nc.scalar.dma_start(wm_sb[:, kt_sl], wm_d.ap()[:, kt_sl])

            # (hidx within packed 512 cols, cos table, sin table, dest)
            norm_specs = [
                (2, ck_sb, sk_sb, kt_sb),
                (0, cq_sb, sq_sb, qt_sb[0]),
                (1, cq_sb, sq_sb, qt_sb[1]),
            ]

            def pass_dma(p):
                """Prefetch the x/mu tiles for projection pass p."""
                xh_t = stream.tile([P, KT, 2, SCP * P], FP8, tag="xh", name="xh_t")
                xl_t = stream.tile([P, KT, 2, SCP * P], FP8, tag="xl", name="xl_t")
                mu_t = stream.tile([P, KT, 2, SCP * P], FP8, tag="mu", name="mu_t")
                nc.sync.dma_start(xh_t[:], xh_d.ap()[p])
                nc.sync.dma_start(xl_t[:], xl_d.ap()[p])
                nc.sync.dma_start(mu_t[:], mu_d.ap()[p])
                return xh_t, xl_t, mu_t

            def proj_pass(p, tiles):
                """Project s-columns [256p, 256p+256): fp8 DoubleRow matmuls,
                then RMSNorm+RoPE+transpose for q0/q1/k, V copy."""
                xh_t, xl_t, mu_t = tiles
                pp = [
                    ps_main.tile([P, 512], F32, tag="main", name=f"pp{i}")
                    for i in range(SCP)
                ]
                for i2 in range(SCP):
                    sl = slice(i2 * P, (i2 + 1) * P)
                    for kt in range(KT):
                        nc.tensor.matmul(
                            pp[i2][:], xh_t[:, kt, :, sl], wh_sb[:, kt],
                            start=(kt == 0), stop=False, perf_mode=DR,
                        )
                        nc.tensor.matmul(
                            pp[i2][:], xl_t[:, kt, :, sl], wh_sb[:, kt],
                            start=False, stop=False, perf_mode=DR,
                        )
                        nc.tensor.matmul(
                            pp[i2][:], xh_t[:, kt, :, sl], wl_sb[:, kt],
                            start=False, stop=False, perf_mode=DR,
                        )
                        nc.tensor.matmul(
                            pp[i2][:], mu_t[:, kt, :, sl], wm_sb[:, kt],
                            start=False, stop=(kt == KT - 1), perf_mode=DR,
                        )
                # normalize + rope + transpose into [d, s] layout
                for i2 in range(SCP):
                    sc = p * SCP + i2
                    ps = pp[i2]
                    for hidx, c_sb, s_sb, dst in norm_specs:
                        off = hidx * P
                        sqv = tmp.tile([P, HEAD_DIM], F32, tag="sqv", name="sqv")
                        var = small.tile([P, 1], F32, tag="var", name="var")
                        nc.scalar.activation(
                            sqv[:], ps[:, off : off + P], AF.Square, accum_out=var[:]
                        )
                        # std64 = sqrt(var/HEAD_DIM + WSCALE^2*eps)
                        #       = WSCALE*sqrt(mean(q^2)+eps); its reciprocal
                        # also cancels the weight pre-scale still in psum
                        std = small.tile([P, 1], F32, tag="std", name="std")
                        nc.scalar.activation(
                            std[:], var[:], AF.Sqrt,
                            scale=1.0 / HEAD_DIM, bias=eps_sb[:],
                        )
                        rstd = small.tile([P, 1], F32, tag="rstd", name="rstd")
                        nc.vector.reciprocal(rstd[:], std[:])
                        t1 = tmp.tile([P, HEAD_DIM], F32, tag="t1", name="t1")
                        nc.vector.scalar_tensor_tensor(
                            t1[:], ps[:, off : off + P], rstd[:], c_sb[:, sc, :],
                            op0=OP.mult, op1=OP.mult,
                        )
                        t2 = tmp.tile([P, HEAD_DIM], F32, tag="t2", name="t2")
                        nc.vector.scalar_tensor_tensor(
                            t2[:, 0:64], ps[:, off + 64 : off + P], rstd[:],
                            s_sb[:, sc, 0:64], op0=OP.mult, op1=OP.mult,
                        )
                        nc.vector.scalar_tensor_tensor(
                            t2[:, 64:P], ps[:, off : off + 64], rstd[:],
                            s_sb[:, sc, 64:P], op0=OP.mult, op1=OP.mult,
                        )
                        qsd = tmp.tile([P, HEAD_DIM], BF16, tag="qsd", name="qsd")
                        nc.vector.tensor_add(qsd[:], t1[:], t2[:])
                        tr = ps_scr.tile([P, P], BF16, tag="scr", name="tr")
                        nc.tensor.transpose(tr[:], qsd[:], ident[:])
                        nc.vector.tensor_copy(dst[:, sc * P : (sc + 1) * P], tr[:])
                    # V: copy (cast) with 1/WSCALE to undo the weight pre-scale
                    nc.scalar.activation(
                        v_sb[:, sc, :], ps[:, 384:512], AF.Copy, scale=1.0 / WSCALE
                    )

            def attention(qc, filler=None):
                """Scores/exp/PV/den for q-chunk qc (columns [512qc,512qc+512)).
                Causal-narrowed; PV lags scores by one kv-block so the PE
                never waits on Exp.  `filler` emits independent PE work (the
                previous chunk's wo matmuls) between kv-blocks to absorb
                latency bubbles."""
                jmax = 4 * qc + 3
                q0 = qc * QCH
                out_ps = [
                    ps_out.tile([P, QCH], F32, tag="out", name=f"out_ps{h}")
                    for h in range(2)
                ]
                den_t = ps_den.tile([64, QCH], F32, tag="den", name="den_t")
                pend = []

                def emit_pv(j, h, e, lo):
                    nc.tensor.matmul(
                        out_ps[h][:, lo:], v_sb[:, j, :], e[:, lo:],
                        start=(j == 0), stop=(j == jmax), skip_group_check=True,
                    )
                    nc.tensor.matmul(
                        den_t[32 * h : 32 * h + 1, lo:], ones_sb[:], e[:, lo:],
                        start=(j == 0), stop=(j == jmax), skip_group_check=True,
                    )

                for j in range(jmax + 1):
                    r = j - 4 * qc
                    lo = max(r, 0) * P
                    for h in range(2):
                        s_ps = ps_main.tile([P, QCH], F32, tag="main", name="s_ps")
                        nc.tensor.matmul(
                            s_ps[:, lo:],
                            kt_sb[:, j * P : (j + 1) * P],
                            qt_sb[h][:, q0 + lo : q0 + QCH],
                            start=True, stop=True,
                        )
                        e = expp.tile([P, QCH], BF16, tag="e", name="e")
                        nc.scalar.activation(
                            e[:, lo:], s_ps[:, lo:], AF.Exp, scale=QK_SCALE
                        )
                        if r >= 0:
                            nc.vector.tensor_mul(
                                e[:, lo : lo + P], e[:, lo : lo + P], mask[:]
                            )
                        pend.append((j, h, e, lo))
                    while len(pend) > 2:
                        emit_pv(*pend.pop(0))
                    if filler is not None:
                        next(filler, None)
                while pend:
                    emit_pv(*pend.pop(0))
                return out_ps, den_t

            def attn_div(qc, out_ps, den_t):
                q_sl = slice(qc * QCH, (qc + 1) * QCH)
                for h in range(2):
                    rd = small.tile([1, QCH], F32, tag="rd", name="rd")
                    nc.vector.reciprocal(rd[:], den_t[32 * h : 32 * h + 1, :])
                    rdb = rdbp.tile([P, QCH], F32, tag="rdb", name="rdb")
                    nc.gpsimd.partition_broadcast(rdb[:], rd[:])
                    nc.vector.tensor_mul(attn_sb[h][:, q_sl], out_ps[h][:], rdb[:])

            def do_wo(qc):
                """Output projection for q chunk qc, as a generator yielding
                after every 2 output chunks; output DMA per 4 chunks."""
                q_sl = slice(qc * QCH, (qc + 1) * QCH)
                ob = ostage.tile([P, KC, QCH], BF16, tag="ob", name="ob")
                for oc in range(KC):
                    o_ps = ps_main.tile([P, QCH], F32, tag="main", name="o_ps")
                    nc.tensor.matmul(
                        o_ps[:], wo_sb[:, 0, oc * P : (oc + 1) * P],
                        attn_sb[0][:, q_sl], start=True, stop=False,
                    )
                    nc.tensor.matmul(
                        o_ps[:], wo_sb[:, 1, oc * P : (oc + 1) * P],
                        attn_sb[1][:, q_sl], start=False, stop=True,
                    )
                    nc.vector.tensor_copy(ob[:, oc, :], o_ps[:])
                    if oc % 4 == 3:
                        g = oc - 3
                        nc.scalar.dma_start(
                            out_d.ap()[g : g + 4, :, q_sl].rearrange(
                                "o p s -> p o s"
                            ),
                            ob[:, g : g + 4, :],
                        )
                    if oc % 2 == 1:
                        yield

            def drain(gen):
                if gen is not None:
                    for _ in gen:
                        pass

            # Phase-separated schedule: all projections, then attention.
            # Keeps the Activation engine on one table set per phase
            # (Square/Sqrt/Copy during projections, Exp during attention)
            # so only 2 table loads happen in the whole program.  The rope
            # tables and wo weights are DMA'd mid-stream, after the early
            # x-tiles they'd otherwise delay.
            for rep in range(repeats):
                tiles = {0: pass_dma(0), 1: pass_dma(1)}
                if rep == 0:
                    # rope tables in small chunks so the DMA-bus FIFO
                    # interleaves them fairly with the x-tile stream
                    pairs = [(cq_sb, cosq), (sq_sb, sinq)]
                    if not shared_rope:
                        pairs += [(ck_sb, cosk), (sk_sb, sink)]
                    for g in range(0, SC, 4):
                        for dst, src in pairs:
                            nc.scalar.dma_start(
                                dst[:, g : g + 4, :],
                                src.ap()[g : g + 4].rearrange("s p d -> p s d"),
                            )
                for p in range(NPASS):
                    if p + 2 < NPASS:
                        tiles[p + 2] = pass_dma(p + 2)
                    proj_pass(p, tiles.pop(p))
                    if p == 4 and rep == 0:
                        nc.scalar.dma_start(
                            wo_sb[:], woT.ap().rearrange("c p o -> p c o"))
                wo_gen = None
                for qc in range(NQC):
                    acc = attention(qc, filler=wo_gen)
                    drain(wo_gen)
                    attn_div(qc, *acc)
                    wo_gen = do_wo(qc)
                drain(wo_gen)
                if dbg:
                    for h in range(2):
                        nc.sync.dma_start(dbg_qt[h].ap(), qt_sb[h][:])
                        nc.sync.dma_start(dbg_at[h].ap(), attn_sb[h][:])
                    nc.sync.dma_start(dbg_kt.ap(), kt_sb[:])
                    nc.sync.dma_start(dbg_v.ap(), v_sb[:])

    nc.compile()
    return nc


def _get_program(repeats=1, shared_rope=True):
    key = (repeats, shared_rope)
    if key not in _PROGRAM:
        _PROGRAM[key] = _build_program(repeats, shared_rope=shared_rope)
    return _PROGRAM[key]


def _pack_x(a):
    """[S, HID] -> [NPASS, P, KT, 2, 256] with hid = kt*256 + i*128 + p."""
    return np.ascontiguousarray(
        a.reshape(NPASS, SCP * P, KT, 2, P).transpose(0, 4, 2, 3, 1)
    )


def _pack_w(a):
    """[HID, 512] -> [P, KT, 2, 512] with hid = kt*256 + i*128 + p."""
    return np.ascontiguousarray(a.reshape(KT, 2, P, 512).transpose(2, 0, 1, 3))


def _host_prepare(inputs):
    """Shard + lay out inputs for the 8 cores."""
    hs = np.asarray(inputs["hidden_states"], dtype=np.float32).reshape(S, HID)
    mu = np.asarray(inputs["mu_prev"], dtype=np.float32).reshape(S, HID)
    wq = np.asarray(inputs["wq"], dtype=np.float32)
    wk = np.asarray(inputs["wk"], dtype=np.float32)
    wv = np.asarray(inputs["wv"], dtype=np.float32)
    wo = np.asarray(inputs["wo"], dtype=np.float32)
    wmq = np.asarray(inputs["wmq"], dtype=np.float32)
    wmk = np.asarray(inputs["wmk"], dtype=np.float32)
    wmv = np.asarray(inputs["wmv"], dtype=np.float32)
    qw = np.asarray(inputs["q_norm_w"], dtype=np.float32)
    kw = np.asarray(inputs["k_norm_w"], dtype=np.float32)

    # hi/lo fp8 split of x; single fp8 for mu (its term is 10x smaller)
    xh8 = hs.astype(NP_E4)
    xl8 = (hs - xh8.astype(np.float32)).astype(NP_E4)
    mu8 = mu.astype(NP_E4)
    xh_p = _pack_x(xh8)
    xl_p = _pack_x(xl8)
    mu_p = _pack_x(mu8)

    # RoPE tables in [s, d] layout with rotate-half sign and norm weight baked in
    inv = 1.0 / (ROPE_THETA ** (np.arange(0, HEAD_DIM, 2, dtype=np.float32) / HEAD_DIM))
    ang = np.arange(S, dtype=np.float32)[:, None] * inv[None, :]  # [S, 64]
    emb = np.concatenate([ang, ang], axis=-1)  # [S, 128]
    cos_e = np.cos(emb)
    sin_e = np.sin(emb)
    sin_s = np.concatenate([-sin_e[:, :64], sin_e[:, 64:]], axis=-1)

    def tables(w):
        w_shift = np.concatenate([w[64:], w[:64]])
        cos_t = (cos_e * w[None, :]).astype(np.float32).reshape(SC, P, HEAD_DIM)
        sin_t = (sin_s * w_shift[None, :]).astype(np.float32).reshape(SC, P, HEAD_DIM)
        return np.ascontiguousarray(cos_t), np.ascontiguousarray(sin_t)

    shared_rope = bool(np.array_equal(qw, kw))
    cq, sq = tables(qw)
    if not shared_rope:
        ck, sk = tables(kw)

    in_maps = []
    for c in range(N_CORES):
        g = c // 2
        wq_s = wq[256 * c : 256 * (c + 1)]      # [256, HID]
        wmq_s = wmq[256 * c : 256 * (c + 1)]
        wk_s = wk[P * g : P * (g + 1)]          # [128, HID]
        wmk_s = wmk[P * g : P * (g + 1)]
        wv_s = wv[P * g : P * (g + 1)]
        wmv_s = wmv[P * g : P * (g + 1)]
        w_all = np.concatenate([wq_s.T, wk_s.T, wv_s.T], axis=1) * WSCALE  # [HID, 512]
        wm_all = np.concatenate([wmq_s.T, wmk_s.T, wmv_s.T], axis=1) * WSCALE
        wh8 = w_all.astype(NP_E4)
        wl8 = (w_all - wh8.astype(np.float32)).astype(NP_E4)
        wm8 = wm_all.astype(NP_E4)
        woT_c = wo[:, 256 * c : 256 * (c + 1)].T                     # [256, HID]
        im = {
            "xh": xh_p,
            "xl": xl_p,
            "mu8": mu_p,
            "wh": _pack_w(wh8),
            "wl": _pack_w(wl8),
            "wm8": _pack_w(wm8),
            "woT": np.ascontiguousarray(woT_c).astype(NP_BF16).reshape(2, P, HID),
            "cosq": cq,
            "sinq": sq,
        }
        if not shared_rope:
            im["cosk"] = ck
            im["sink"] = sk
        in_maps.append(im)
    return in_maps, shared_rope


def run(inputs, trace=False):
    """Run the SPMD kernel; returns (full_output, exec_time_ns_or_None)."""
    in_maps, shared_rope = _host_prepare(inputs)
    nc = _get_program(shared_rope=shared_rope)
    res = run_bass_kernel_spmd(
        nc, in_maps, core_ids=list(range(N_CORES)), trace=trace
    )
    total = np.zeros((HID, S), dtype=np.float32)
    for c in range(N_CORES):
        total += res.results[c]["out"].astype(np.float32).reshape(HID, S)
    out = np.ascontiguousarray(total.T).reshape(B, S, HID).astype(np.float32)
    return out, res.exec_time_ns


def kernel(**inputs) -> np.ndarray:
    out, _ = run(inputs, trace=False)
    return out
